# revision 1
# baseline (speedup 1.0000x reference)
"""CKANKANNet Trainium2 kernel builder (per-core SPMD program, B=8 samples/core).

Algorithm: v = 2.5*x + 5.5 maps the spline grid to integer knots;
b_j(x) = N3(v-j) = (1/6) * 4th finite difference over m of relu(v-m)^3,
with x clamped at 2.2 (beyond the grid all bases are exactly 0) so cubes
stay <= 1331 and the fp32 differences are accurate. /6 folded into weights.
Convs: bf16 matmuls, channels on K partitions, 3x3 taps as accumulating
matmuls with edge-trimmed N ranges (no spatial padding).
L1 bakes ky taps into K via 3 shifted channel-block copies (K=81).
"""
import sys
sys.path.insert(0, '/opt/trn_rl_repo')
from contextlib import ExitStack

import numpy as np
import ml_dtypes
MM_NP = np.float16

import concourse.bass as bass
import concourse.tile as tile
from concourse import bacc, mybir

F32 = mybir.dt.float32
F16 = mybir.dt.float16
BF16 = mybir.dt.bfloat16  # unused for operands now
MMDT = mybir.dt.float16
AF = mybir.ActivationFunctionType
OP = mybir.AluOpType

B = 8
NB = 8
NM = 12
O_OUT = 100


def silu_np(x):
    return x / (1.0 + np.exp(-x))


def fold_weights(wb1, ws1, wb2, ws2, wb3, ws3, lb, lc):
    out = {}
    W1 = np.zeros((96, 3 * 64), np.float32)
    for kyi in range(3):
        for kxi in range(3):
            W1[kyi * 32 + 0:kyi * 32 + 3, kxi * 64:(kxi + 1) * 64] = wb1[:, :, kyi, kxi].T
            blk = np.transpose(ws1[:, :, kyi, kxi].reshape(64, 3, NB), (2, 1, 0)) / 6.0
            W1[kyi * 32 + 3:kyi * 32 + 27, kxi * 64:(kxi + 1) * 64] = blk.reshape(24, 64)
    out['w1'] = W1.astype(MM_NP)

    W2 = np.zeros((576, 9 * 128), np.float32)
    for kyi in range(3):
        for kxi in range(3):
            t = kyi * 3 + kxi
            blk = np.transpose(ws2[:, :, kyi, kxi].reshape(128, 64, NB), (2, 1, 0)) / 6.0
            W2[0:512, t * 128:(t + 1) * 128] = blk.reshape(512, 128)
            W2[512:576, t * 128:(t + 1) * 128] = wb2[:, :, kyi, kxi].T
    out['w2'] = W2.astype(MM_NP)

    W3 = np.zeros((1152, 9 * 64), np.float32)
    for kyi in range(3):
        for kxi in range(3):
            t = kyi * 3 + kxi
            blk = np.transpose(ws3[:, :, kyi, kxi].reshape(64, 128, NB), (2, 1, 0)) / 6.0
            W3[0:1024, t * 64:(t + 1) * 64] = blk.reshape(1024, 64)
            W3[1024:1152, t * 64:(t + 1) * 64] = wb3[:, :, kyi, kxi].T
    out['w3'] = W3.astype(MM_NP)

    lc_r = lc.reshape(O_OUT, 64, 64, NB)
    lb_r = lb.reshape(O_OUT, 64, 64)
    WL = np.zeros((64, 4, 14400), np.float32)
    for p in range(4):
        for yi in range(16):
            yx = p * 16 + yi
            for j in range(NB):
                WL[:, p, (yi * NB + j) * O_OUT:(yi * NB + j + 1) * O_OUT] = \
                    lc_r[:, :, yx, j].T / 6.0
            WL[:, p, 12800 + yi * O_OUT:12800 + (yi + 1) * O_OUT] = lb_r[:, :, yx].T
    out['wl'] = WL.reshape(64, 4 * 14400).astype(MM_NP)
    return out


def emit_basis(nc, bpool, tspool, src_ap, P, E, d3_consumer, ck, bias_tiles):
    """src_ap: [P, E] view of activations. cube slabs + d1..d3;
    d3_consumer(D3, E, ck) emits the 4th diff. Slot reuse: d2->c3, d3->d1.
    Clamp x<=2.2 folded into ACT: rx = relu(2.2-x); t_m = relu(11-m - 2.5*rx)."""
    RX = tspool.tile([P, E], F32, tag="vc")
    nc.scalar.activation(RX[:], src_ap, AF.Relu, bias=bias_tiles['c22'][0:P, :],
                         scale=-1.0)
    C3 = bpool.tile([P, NM * E], F32, tag="c3")
    for m in range(NM):
        T = tspool.tile([P, E], F32, tag="bt")
        S = tspool.tile([P, E], F32, tag="bs")
        nc.scalar.activation(T[:], RX[:], AF.Relu, bias=bias_tiles[m][0:P, :],
                             scale=-2.5)
        nc.scalar.activation(S[:], T[:], AF.Square)
        nc.vector.tensor_tensor(C3[:, m * E:(m + 1) * E], T[:], S[:], op=OP.mult)
    D1 = bpool.tile([P, (NM - 1) * E], F32, tag="d1", bufs=1)
    nc.vector.tensor_tensor(D1[:], C3[:, 0:(NM - 1) * E], C3[:, E:NM * E], op=OP.subtract)
    D2 = bpool.tile([P, (NM - 2) * E], F32, tag="c3", name="D2")
    nc.vector.tensor_tensor(D2[:], D1[:, 0:(NM - 2) * E], D1[:, E:(NM - 1) * E], op=OP.subtract)
    D3 = bpool.tile([P, (NM - 3) * E], F32, tag="d1", name="D3", bufs=1)
    nc.vector.tensor_tensor(D3[:], D2[:, 0:(NM - 3) * E], D2[:, E:(NM - 2) * E], op=OP.subtract)
    d3_consumer(D3, E, ck)


def maxpool_from_psum(nc, tmppool, psum_ap, P, n_bh, W_half, out_ap):
    W = 2 * W_half
    pv = psum_ap.rearrange("p (hp r2 wp c2) -> p hp wp r2 c2",
                           hp=n_bh, r2=2, wp=W_half, c2=2)
    nc.vector.tensor_reduce(out_ap.rearrange("p (hp wp) -> p hp wp", wp=W_half),
                            pv, mybir.AxisListType.XY, OP.max, opt_input=False)


def build_nc(dbg=()):
    nc = bacc.Bacc("TRN2", target_bir_lowering=False, debug=False, num_devices=8)
    x_ext = nc.declare_dram_parameter("x", [B, 3, 64, 64], F32, isOutput=False)
    w1_ext = nc.declare_dram_parameter("w1", [96, 192], MMDT, isOutput=False)
    w2_ext = nc.declare_dram_parameter("w2", [576, 1152], MMDT, isOutput=False)
    w3_ext = nc.declare_dram_parameter("w3", [1152, 576], MMDT, isOutput=False)
    wl_ext = nc.declare_dram_parameter("wl", [64, 57600], MMDT, isOutput=False)
    out_ext = nc.declare_dram_parameter("out", [B, O_OUT], F32, isOutput=True)

    dbg_exts = {}

    def dbg_tap(name, shape, dt=F32):
        if name in dbg:
            dbg_exts[name] = nc.declare_dram_parameter(f"dbg_{name}", shape, dt, isOutput=True)
            return dbg_exts[name]
        return None

    with tile.TileContext(nc) as tc, ExitStack() as ctx:
        persist = ctx.enter_context(tc.tile_pool(name="persist", bufs=1))
        wpool = ctx.enter_context(tc.tile_pool(name="wpool", bufs=1))
        bpool = ctx.enter_context(tc.tile_pool(name="bpool", bufs=2))
        tspool = ctx.enter_context(tc.tile_pool(name="tspool", bufs=2))
        d4pool = ctx.enter_context(tc.tile_pool(name="d4pool", bufs=2))
        tmppool = ctx.enter_context(tc.tile_pool(name="tmppool", bufs=2))

        bias_tiles = {}
        for m in range(NM):
            bt_m = wpool.tile([128, 1], F32, tag=f"bias_{m}", name=f"bias{m}")
            nc.gpsimd.memset(bt_m[:], float(11 - m))
            bias_tiles[m] = bt_m
        bt_c = wpool.tile([128, 1], F32, tag="bias_c22", name="biasc22")
        nc.gpsimd.memset(bt_c[:], 2.2)
        bias_tiles['c22'] = bt_c
        w1sb = wpool.tile([96, 192], MMDT)
        nc.sync.dma_start(w1sb[:], w1_ext.ap())
        zt = wpool.tile([128, 2064], MMDT)
        nc.gpsimd.memset(zt[:], 0.0)

        h1 = persist.tile([64, 8192], F16)
        h2 = persist.tile([128, 2048], F32)
        h3 = persist.tile([64, 512], F32)

        # ================= L1 =================
        with tc.tile_pool(name="l1pool", bufs=1) as l1p:
            X1 = l1p.tile([128, 768], F32)
            for c in range(3):
                nc.sync.dma_start(
                    X1[:, c * 256:(c + 1) * 256],
                    x_ext.ap()[:, c, :, :].rearrange("b (g hh) w -> b g (hh w)", g=16))
            sl1 = l1p.tile([128, 768], MMDT)
            nc.scalar.activation(sl1[:], X1[:], AF.Silu)

            D4_1 = l1p.tile([128, NB * 768], MMDT)
            d41_v = D4_1[:].rearrange("p (j e) -> p j e", j=NB)

            def d3c_l1(D3, E, ck):
                nc.vector.tensor_tensor(
                    d41_v[:, :, ck * E:(ck + 1) * E],
                    D3[:, 0:NB * E].rearrange("p (j e) -> p j e", j=NB),
                    D3[:, E:(NB + 1) * E].rearrange("p (j e) -> p j e", j=NB),
                    op=OP.subtract)
            for ck in range(2):
                emit_basis(nc, bpool, tspool,
                           X1[:, ck * 384:(ck + 1) * 384], 128, 384, d3c_l1, ck, bias_tiles)

            if (t := dbg_tap('d41', [128, NB * 768], MMDT)) is not None:
                nc.sync.dma_start(t.ap(), D4_1[:])

            # dump channels to DRAM (ch-major), then read back partition-
            # parallel into the three shifted ky blocks (32-aligned, K=96).
            l1ch = nc.dram_tensor("l1ch", [27, B * 4096], MMDT)
            for c in range(3):
                nc.sync.dma_start(
                    l1ch.ap()[c, :].rearrange("(bg e) -> bg e", e=256),
                    sl1[:, c * 256:(c + 1) * 256])
            for j in range(NB):
                nc.sync.dma_start(
                    l1ch.ap()[3 + j * 3:3 + j * 3 + 3, :]
                        .rearrange("c (bg e) -> bg c e", e=256),
                    D4_1[:, j * 768:(j + 1) * 768]
                        .rearrange("p (c e) -> p c e", e=256))

            BH = 4  # images per half
            for bh in range(2):
                Bun1 = l1p.tile([96, 64 + BH * 4096 + 64], MMDT, tag="bun1", name="Bun1")
                for kyi in range(3):
                    base = 64 + (1 - kyi) * 64
                    nc.sync.dma_start(
                        Bun1[kyi * 32:kyi * 32 + 27, base:base + BH * 4096],
                        l1ch.ap()[:, bh * BH * 4096:(bh + 1) * BH * 4096])
                    # dead K rows (27..31 of each block): zero
                    for k in range(8):
                        nc.sync.dma_start(
                            Bun1[kyi * 32 + 27:kyi * 32 + 32,
                                 k * 2064:(k + 1) * 2064],
                            zt[0:5, :])
                # boundary rows: ky=0 block box-row 0; ky=2 block box-row 63
                for b in range(BH):
                    nc.sync.dma_start(
                        Bun1[0:27, 64 + b * 4096: 64 + b * 4096 + 64],
                        zt[0:27, 0:64])
                    nc.sync.dma_start(
                        Bun1[64:91, 64 + b * 4096 + 63 * 64: 64 + b * 4096 + 64 * 64],
                        zt[0:27, 0:64])

                bun1_v = Bun1[:, 64:64 + BH * 4096].rearrange(
                    "p (b r w) -> p b r w", b=BH, w=64)
                chunks = [(bi, hb) for bi in range(BH) for hb in range(8)]
                with tc.tile_pool(name="pp1", bufs=1, space="PSUM") as pp1:
                    for g in range(0, len(chunks), 8):
                        grp = chunks[g:g + 8]
                        pss = [pp1.tile([64, 512], F32, tag=f"ps1_{i}", name=f"ps1_{i}")
                               for i in range(len(grp))]
                        for ti, kxi in enumerate([1, 0, 2]):
                            for ci, (bi, hb) in enumerate(grp):
                                ps = pss[ci]
                                if kxi == 0:
                                    mv = bun1_v[:, bi, hb * 8:hb * 8 + 8, 0:63]
                                    ov = ps[:].rearrange("p (r w) -> p r w", w=64)[:, :, 1:64]
                                elif kxi == 1:
                                    mv = bun1_v[:, bi, hb * 8:hb * 8 + 8, :]
                                    ov = ps[:]
                                else:
                                    mv = bun1_v[:, bi, hb * 8:hb * 8 + 8, 1:64]
                                    ov = ps[:].rearrange("p (r w) -> p r w", w=64)[:, :, 0:63]
                                nc.tensor.matmul(ov, w1sb[:, kxi * 64:(kxi + 1) * 64], mv,
                                                 start=(ti == 0), stop=(ti == 2))
                        for ci, (bi, hb) in enumerate(grp):
                            b = bh * BH + bi
                            maxpool_from_psum(nc, tmppool, pss[ci][:], 64, 4, 32,
                                              h1[:, b * 1024 + hb * 128: b * 1024 + (hb + 1) * 128])
        if (t := dbg_tap('h1', [64, 8192], F16)) is not None:
            nc.sync.dma_start(t.ap(), h1[:])

        # ================= L2 =================
        with tc.tile_pool(name="l2pool", bufs=1) as l2p:
            w2sb = [l2p.tile([128 if i < 4 else 64, 1152], MMDT, tag=f"w2_{i}",
                             name=f"w2sb{i}") for i in range(5)]
            for i in range(5):
                nc.sync.dma_start(w2sb[i][:], w2_ext.ap()[i * 128:min(576, (i + 1) * 128), :])
            h1s = l2p.tile([128, 4096], F16)
            nc.sync.dma_start(h1s[0:64, :], h1[:, 0:4096])
            nc.sync.dma_start(h1s[64:128, :], h1[:, 4096:8192])

            # h1s partition layout: rows 0..63 = channels for images 0..3,
            # rows 64..127 = channels for images 4..7 (column-aligned pixels).
            # Half bh processes h1s columns [bh*2048, (bh+1)*2048): images
            # (bh*2, bh*2+1) in rows<64 and (4+bh*2, 4+bh*2+1) in rows>=64.
            for bh in range(2):
                imgs = [bh * 2, bh * 2 + 1, 4 + bh * 2, 4 + bh * 2 + 1]
                Ts2 = l2p.tile([64, 4096], MMDT, tag="ts2", name="Ts2")
                # silu for the 4 images of this half: h1 columns are b-major
                for ii, b in enumerate(imgs):
                    nc.scalar.activation(Ts2[:, ii * 1024:(ii + 1) * 1024],
                                         h1[:, b * 1024:(b + 1) * 1024], AF.Silu)
                T2 = [l2p.tile([128, 4096], MMDT, tag=f"t2_{i}", name=f"T2_{i}")
                      for i in range(4)]

                def d3c_l2(D3, E, ck, T2=T2):
                    D4 = d4pool.tile([128, NB * E], MMDT, tag="d4_l2", name="D4")
                    nc.vector.tensor_tensor(D4[:], D3[:, 0:NB * E], D3[:, E:(NB + 1) * E],
                                            op=OP.subtract)
                    # ck covers cols [ck*E,(ck+1)*E) of the half's 2048-col
                    # window; image-in-window ii = ph*2 + ck//2 (E=512)
                    for ph in range(2):
                        ii = ph * 2 + ck // 2
                        off = ii * 1024 + (ck % 2) * E
                        for j in range(NB):
                            nc.sync.dma_start(
                                T2[j // 2][(j % 2) * 64:(j % 2) * 64 + 64,
                                           off:off + E],
                                D4[ph * 64:(ph + 1) * 64, j * E:(j + 1) * E])
                for ck in range(4):
                    emit_basis(nc, bpool, tspool,
                               h1s[:, bh * 2048 + ck * 512: bh * 2048 + (ck + 1) * 512],
                               128, 512, d3c_l2, ck, bias_tiles)

                t2v = [T2[i][:].rearrange("p (b h w) -> p b h w", b=4, w=32)
                       for i in range(4)]
                ts2v = Ts2[:].rearrange("p (b h w) -> p b h w", b=4, w=32)
                taps = [(0, 1, 1)] + [(kt, kyi, kxi) for kt in range(5)
                                      for kyi in range(3) for kxi in range(3)
                                      if (kt, kyi, kxi) != (0, 1, 1)]
                n_taps = len(taps)
                chunks = [(ii, half) for ii in range(4) for half in range(2)]
                with tc.tile_pool(name="pp2", bufs=1, space="PSUM") as pp2:
                    pss = [pp2.tile([128, 512], F32, tag=f"ps2_{i}", name=f"ps2_{i}")
                           for i in range(8)]
                    for tapi, (kt, kyi, kxi) in enumerate(taps):
                        for ci, (ii, half) in enumerate(chunks):
                            h0 = half * 16
                            ps = pss[ci]
                            r_lo = max(0, 1 - kyi - h0)
                            r_hi = min(16, 33 - h0 - kyi)
                            w_lo = 1 if kxi == 0 else 0
                            w_hi = 31 if kxi == 2 else 32
                            in_row = h0 + r_lo + kyi - 1
                            in_col = w_lo + kxi - 1
                            src = t2v[kt] if kt < 4 else ts2v
                            mv = src[:, ii, in_row:in_row + (r_hi - r_lo),
                                     in_col:in_col + (w_hi - w_lo)]
                            ov = ps[:].rearrange("p (r w) -> p r w", w=32)[
                                :, r_lo:r_hi, w_lo:w_hi]
                            nc.tensor.matmul(
                                ov,
                                w2sb[kt][:, (kyi * 3 + kxi) * 128:
                                         (kyi * 3 + kxi + 1) * 128],
                                mv, start=(tapi == 0),
                                stop=(tapi == n_taps - 1))
                    for ci, (ii, half) in enumerate(chunks):
                        b = imgs[ii]
                        maxpool_from_psum(nc, tmppool, pss[ci][:], 128, 8, 16,
                                          h2[:, b * 256 + half * 128:
                                             b * 256 + (half + 1) * 128])
        if (t := dbg_tap('h2', [128, 2048])) is not None:
            nc.sync.dma_start(t.ap(), h2[:])

        # ================= L3 =================
        with tc.tile_pool(name="l3pool", bufs=1) as l3p:
            w3sb = [l3p.tile([128, 576], MMDT, tag=f"w3_{i}", name=f"w3sb{i}")
                    for i in range(9)]
            for i in range(9):
                nc.sync.dma_start(w3sb[i][:], w3_ext.ap()[i * 128:(i + 1) * 128, :])
            for bh in range(2):
                Ts3 = l3p.tile([128, 1024], MMDT, tag="ts3", name="Ts3")
                nc.scalar.activation(Ts3[:], h2[:, bh * 1024:(bh + 1) * 1024], AF.Silu)
                T3 = [l3p.tile([128, 1024], MMDT, tag=f"t3_{j}", name=f"T3_{j}")
                      for j in range(NB)]

                def d3c_l3(D3, E, ck, T3=T3):
                    for j in range(NB):
                        nc.vector.tensor_tensor(
                            T3[j][:, ck * 512:(ck + 1) * 512],
                            D3[:, j * E:(j + 1) * E], D3[:, (j + 1) * E:(j + 2) * E],
                            op=OP.subtract)
                for ck in range(2):
                    emit_basis(nc, bpool, tspool,
                               h2[:, bh * 1024 + ck * 512: bh * 1024 + (ck + 1) * 512],
                               128, 512, d3c_l3, ck, bias_tiles)

                t3v = [T3[j][:].rearrange("p (b h w) -> p b h w", b=4, w=16)
                       for j in range(NB)]
                ts3v = Ts3[:].rearrange("p (b h w) -> p b h w", b=4, w=16)
                taps = [(0, 1, 1)] + [(kt, kyi, kxi) for kt in range(9)
                                      for kyi in range(3) for kxi in range(3)
                                      if (kt, kyi, kxi) != (0, 1, 1)]
                n_taps = len(taps)
                with tc.tile_pool(name="pp3", bufs=1, space="PSUM") as pp3:
                    pss = [pp3.tile([64, 512], F32, tag=f"ps3_{i}", name=f"ps3_{i}")
                           for i in range(2)]
                    for tapi, (kt, kyi, kxi) in enumerate(taps):
                        for ckc in range(2):
                            b0 = ckc * 2
                            ps = pss[ckc]
                            r_lo = max(0, 1 - kyi)
                            r_hi = min(16, 17 - kyi)
                            w_lo = 1 if kxi == 0 else 0
                            w_hi = 15 if kxi == 2 else 16
                            src = t3v[kt] if kt < 8 else ts3v
                            mv = src[:, b0:b0 + 2, r_lo + kyi - 1:r_hi + kyi - 1,
                                     w_lo + kxi - 1:w_lo + kxi - 1 + (w_hi - w_lo)]
                            ov = ps[:].rearrange("p (b r w) -> p b r w", b=2, w=16)[
                                :, :, r_lo:r_hi, w_lo:w_hi]
                            nc.tensor.matmul(
                                ov,
                                w3sb[kt][:, (kyi * 3 + kxi) * 64:
                                         (kyi * 3 + kxi + 1) * 64],
                                mv, start=(tapi == 0),
                                stop=(tapi == n_taps - 1))
                    for ckc in range(2):
                        maxpool_from_psum(nc, tmppool, pss[ckc][:], 64, 16, 8,
                                          h3[:, (bh * 2 + ckc) * 128:(bh * 2 + ckc + 1) * 128])
        if (t := dbg_tap('h3', [64, 512])) is not None:
            nc.sync.dma_start(t.ap(), h3[:])

        # ================= Linear =================
        sl3 = persist.tile([64, 512], MMDT)
        nc.scalar.activation(sl3[:], h3[:], AF.Silu)
        D4L = persist.tile([64, NB * 512], MMDT)

        d4l_vv = D4L[:].rearrange("p (j e) -> p j e", j=NB)

        def d3c_lin(D3, E, ck):
            nc.vector.tensor_tensor(
                d4l_vv[:, :, ck * E:(ck + 1) * E],
                D3[:, 0:NB * E].rearrange("p (j e) -> p j e", j=NB),
                D3[:, E:(NB + 1) * E].rearrange("p (j e) -> p j e", j=NB),
                op=OP.subtract)
        emit_basis(nc, bpool, tspool, h3[:], 64, 512, d3c_lin, 0, bias_tiles)
        if (t := dbg_tap('d4l', [64, NB * 512], MMDT)) is not None:
            nc.sync.dma_start(t.ap(), D4L[:])

        with tc.tile_pool(name="wlpool", bufs=2) as wlp, \
                tc.tile_pool(name="ppl", bufs=1, space="PSUM") as plin:
            psl = plin.tile([B, O_OUT], F32)
            d4l_v = D4L[:].rearrange("p (j b yx) -> p j b yx", j=NB, b=B)
            sl3_v = sl3[:].rearrange("p (b yx) -> p b yx", b=B)
            first = True
            for piece in range(4):
                wlt = wlp.tile([64, 14400], MMDT, tag="wl_piece", name="wlt")
                nc.sync.dma_start(wlt[:], wl_ext.ap()[:, piece * 14400:(piece + 1) * 14400])
                for yi in range(16):
                    yx = piece * 16 + yi
                    for j in range(NB):
                        nc.tensor.matmul(
                            psl[:], d4l_v[:, j, :, yx],
                            wlt[:, (yi * NB + j) * O_OUT:(yi * NB + j + 1) * O_OUT],
                            start=first, stop=False)
                        first = False
                    nc.tensor.matmul(
                        psl[:], sl3_v[:, :, yx],
                        wlt[:, 12800 + yi * O_OUT:12800 + (yi + 1) * O_OUT],
                        start=False, stop=(piece == 3 and yi == 15))
            osb = persist.tile([B, O_OUT], F32)
            nc.vector.tensor_copy(osb[:], psl[:])
            nc.sync.dma_start(out_ext.ap(), osb[:])

    nc.compile()
    return nc

# ===================================================================== runner
from concourse.bass_utils import run_bass_kernel_spmd

_NC_CACHE = {}


def _get_nc():
    if 'nc' not in _NC_CACHE:
        _NC_CACHE['nc'] = build_nc(dbg=())
    return _NC_CACHE['nc']


def kernel(x, wb1, ws1, wb2, ws2, wb3, ws3, lb, lc):
    """Full-input entry point: x [64,3,64,64] f32 -> out [64,100] f32.
    Shards the batch over 8 NeuronCores (8 samples each), replicating weights."""
    x = np.ascontiguousarray(np.asarray(x, dtype=np.float32))
    w = fold_weights(np.asarray(wb1, np.float32), np.asarray(ws1, np.float32),
                     np.asarray(wb2, np.float32), np.asarray(ws2, np.float32),
                     np.asarray(wb3, np.float32), np.asarray(ws3, np.float32),
                     np.asarray(lb, np.float32), np.asarray(lc, np.float32))
    nc = _get_nc()
    in_maps = [{'x': x[i * B:(i + 1) * B], **w} for i in range(8)]
    res = run_bass_kernel_spmd(nc, in_maps, core_ids=list(range(8)))
    return np.concatenate([res.results[i]['out'] for i in range(8)], axis=0)

